# revision 13
# baseline (speedup 1.0000x reference)
"""Trainium2 Bass kernel for nn_Decoder (LSTM decoder + fc1/relu/fc2 head).

Strategy (8 NeuronCores, data-parallel over batch):
  - Each core runs the 511-step LSTM recurrence for its 32-row batch shard.
    Per step, gates[32, 2048] = hT.T @ W_resh (+ x*w_ih + bias via a K=2
    aux matmul whose lhsT rows are [trg_t; ones], DMA'd per step).
    Matmuls use float32r (full-rate fp32 on the PE for N>=256).
    Gate columns are interleaved per 512-col PSUM bank as [i_b f_b o_b g_b]
    so each bank maps to one 128-wide h-chunk.
    Activations (sigmoid table only; tanh(x) = 2*sigmoid(2x)-1) evict
    PSUM->SBUF, then PE transposes move activated gates into transposed
    space [128, 32] per chunk, where the c/h updates run on full 128
    partitions and directly produce the next step's lhsT (hT) slices.
  - Head: fc1+relu into z[32,1024], PE-transpose to zT, then fc2 streams
    the 131MB fc2_w.T from HBM in [128, 2000] tiles (memory-bound).
  - Host side: shard batch, pre-permute weights, concat per-core outputs.
"""

import sys

sys.path.insert(0, "/opt/trn_rl_repo")

import numpy as np
from contextlib import ExitStack

import concourse.bass as bass
import concourse.mybir as mybir
import concourse.tile as tile
from concourse.bass_utils import run_bass_kernel_spmd
from concourse.masks import make_identity
from concourse.vector_clock import ScopedClock

F32 = mybir.dt.float32
F32R = mybir.dt.float32r
AFT = mybir.ActivationFunctionType


def _r(ap):
    """Matmul operands are declared float32r natively; identity passthrough."""
    return ap

N_CORES = 8
B = 256
BSH = B // N_CORES  # 32 batch rows per core
H = 512
G = 4 * H  # 2048
HID = 1024
V = 32000
T_STEPS = 511  # LSTM consumes trg[:, 0:511]

NW = 2000  # fc2 vocab window
NBANK = 500  # fc2 psum bank width (4 banks per window)
N_WIN = V // NW  # 16


_MAX_WAITS = 1


def _split_multi_waits(nc):
    """This container's walrus accepts at most one sync-wait per TPB
    instruction. Move extra waits onto same-engine nops placed directly
    before the instruction (engines execute their stream in order, so
    this is semantically identical)."""
    ctr = 0
    for fn in nc.m.functions:
        for bb in fn.blocks:
            insts = list(bb.instructions)
            out = []
            changed = False
            for inst in insts:
                si = inst.sync_info
                if si is not None and si.on_wait and len(si.on_wait) > _MAX_WAITS:
                    waits = list(si.on_wait)
                    for w in waits[:-_MAX_WAITS]:
                        ctr += 1
                        nop = mybir.InstNoOp(
                            name=f"swsplit-{ctr}",
                            engine=inst.engine,
                            bass_nofuse=True,
                            sync_info=mybir.SyncInfo(on_wait=[w], on_update=[]),
                        )
                        nc.register_instruction(nop, overwrite=True)
                        out.append(nop)
                    si.on_wait = waits[-_MAX_WAITS:]
                    changed = True
                out.append(inst)
            if changed:
                bb.instructions = out


class _SplitDrainTileContext(tile.TileContext):
    def schedule_and_allocate(self):
        ret = super().schedule_and_allocate()
        _split_multi_waits(self.nc)
        return ret


def _build_program(n_steps=T_STEPS):
    nc = bass.Bass("TRN2", target_bir_lowering=False, debug=False, num_devices=1)

    # DRAM I/O
    wt_d = nc.dram_tensor("wt", [4, 128, G], F32R, kind="ExternalInput").ap()
    wib_d = nc.dram_tensor("wib", [2, G], F32R, kind="ExternalInput").ap()
    trga_d = nc.dram_tensor("trga", [max(n_steps, 1), 2, BSH], F32R,
                            kind="ExternalInput").ap()
    fc1t_d = nc.dram_tensor("fc1t", [4, 128, HID], F32R, kind="ExternalInput").ap()
    fc1b_d = nc.dram_tensor("fc1b", [1, HID], F32R, kind="ExternalInput").ap()
    fc2t_d = nc.dram_tensor("fc2t", [8, 128, V], F32R, kind="ExternalInput").ap()
    fc2b_d = nc.dram_tensor("fc2b", [1, V], F32R, kind="ExternalInput").ap()
    ident_d = nc.dram_tensor("ident", [32, 32], F32R, kind="ExternalInput").ap()
    ones_d = nc.dram_tensor("onesr", [1, BSH], F32R, kind="ExternalInput").ap()
    zi_d = nc.dram_tensor("zi", [128, 128], F32R, kind="ExternalInput").ap()
    out_d = nc.dram_tensor("out", [BSH, V], F32, kind="ExternalOutput").ap()

    with _SplitDrainTileContext(nc) as tc, ExitStack() as ctx:
        const = ctx.enter_context(tc.tile_pool(name="const", bufs=1))
        state = ctx.enter_context(tc.tile_pool(name="state", bufs=1))
        work = ctx.enter_context(tc.tile_pool(name="work", bufs=1))
        auxp = ctx.enter_context(tc.tile_pool(name="auxp", bufs=1))

        # resident weights
        wt = const.tile([128, 4 * G], F32R)  # 4 k-chunks side by side
        for kc in range(4):
            nc.sync.dma_start(wt[:, kc * G:(kc + 1) * G], wt_d[kc])
        wib = const.tile([2, G], F32R)
        nc.sync.dma_start(wib[:], wib_d[:])
        fc1t = const.tile([128, 4 * HID], F32R)
        for kc in range(4):
            nc.sync.dma_start(fc1t[:, kc * HID:(kc + 1) * HID], fc1t_d[kc])
        fc1b = const.tile([1, HID], F32R)
        nc.sync.dma_start(fc1b[:], fc1b_d[:])
        ident = const.tile([32, 32], F32R)
        nc.sync.dma_start(ident[:], ident_d[:])
        ones = const.tile([1, BSH], F32R)
        nc.sync.dma_start(ones[:], ones_d[:])

        # LSTM state (transposed space): col-slice b = chunk b ([128 h] x [32 batch])
        cT = state.tile([128, 128], F32)
        hT = state.tile([128, 128], F32R)
        nc.vector.memset(cT[:], 0.0)
        nc.sync.dma_start(hT[:], zi_d[:])

        # per-step work tiles
        acts = work.tile([BSH, G], F32R)     # activated gates, [32, 2048]
        tg = work.tile([128, 128], F32)     # tanh(g) chunks
        t1 = work.tile([128, 128], F32)     # i*tanh(g)
        sc = work.tile([128, 128], F32)     # sigmoid(2c)
        tc_ = work.tile([128, 128], F32)    # tanh(c)
        aux = [auxp.tile([2, BSH], F32R, tag=f"aux{p}", name=f"aux{p}")
               for p in range(2)]

        with tc.tile_pool(name="psum_g", bufs=1, space="PSUM") as pg_pool, \
             tc.tile_pool(name="psum_t", bufs=1, space="PSUM") as pt_pool:
            pg = pg_pool.tile([BSH, G], F32)      # 4 banks, rows 0-31
            pT = pt_pool.tile([128, 512], F32R)  # transposed gates, 1 bank

            # prefetch aux for step 0
            nc.sync.dma_start(aux[0][:], trga_d[0])

            for t in range(n_steps):
                if t + 1 < n_steps:
                    nc.sync.dma_start(aux[(t + 1) % 2][:], trga_d[t + 1])
                # gates matmuls, bank-major
                for b in range(4):
                    bank = pg[:, b * 512:(b + 1) * 512]
                    for kc in range(4):
                        nc.tensor.matmul(
                            bank,
                            lhsT=_r(hT[:, kc * 32:(kc + 1) * 32]),
                            rhs=_r(wt[:, kc * G + b * 512:
                                      kc * G + (b + 1) * 512]),
                            start=(kc == 0), stop=False)
                    nc.tensor.matmul(
                        bank, lhsT=_r(aux[t % 2][:]),
                        rhs=_r(wib[:, b * 512:(b + 1) * 512]),
                        start=False, stop=True)

                # activations: evict PSUM -> SBUF (sigmoid table only)
                for b in range(4):
                    nc.scalar.activation(
                        acts[:, b * 512: b * 512 + 384],
                        pg[:, b * 512: b * 512 + 384], AFT.Sigmoid)
                    nc.scalar.activation(
                        acts[:, b * 512 + 384: b * 512 + 512],
                        pg[:, b * 512 + 384: b * 512 + 512], AFT.Sigmoid,
                        scale=2.0)

                # transpose activated gates into pT: per bank b ->
                # [iT fT oT sgT] at cols b*... (iT_b at 0+b*32 etc.)
                for b in range(4):
                    for x in range(4):
                        nc.tensor.transpose(
                            _r(pT[:, x * 128 + b * 32: x * 128 + (b + 1) * 32]),
                            _r(acts[:, b * 512 + x * 128:
                                    b * 512 + (x + 1) * 128]),
                            _r(ident[:]))

                # transposed-space cell/state update, per chunk b
                for b in range(4):
                    sl = slice(b * 32, (b + 1) * 32)
                    iT = pT[:, 0 * 128 + b * 32: 0 * 128 + (b + 1) * 32]
                    fT = pT[:, 1 * 128 + b * 32: 1 * 128 + (b + 1) * 32]
                    oT = pT[:, 2 * 128 + b * 32: 2 * 128 + (b + 1) * 32]
                    sgT = pT[:, 3 * 128 + b * 32: 3 * 128 + (b + 1) * 32]
                    # tg = 2*sg - 1
                    nc.vector.tensor_scalar(tg[:, sl], sgT, 2.0, 1.0,
                                            mybir.AluOpType.mult,
                                            mybir.AluOpType.subtract)
                    nc.vector.tensor_mul(t1[:, sl], iT, tg[:, sl])
                    nc.vector.tensor_mul(cT[:, sl], cT[:, sl], fT)
                    nc.vector.tensor_add(cT[:, sl], cT[:, sl], t1[:, sl])
                    nc.scalar.activation(sc[:, sl], cT[:, sl], AFT.Sigmoid,
                                         scale=2.0)
                    nc.vector.tensor_scalar(tc_[:, sl], sc[:, sl], 2.0, 1.0,
                                            mybir.AluOpType.mult,
                                            mybir.AluOpType.subtract)
                    nc.vector.tensor_mul(hT[:, sl], oT, tc_[:, sl])

        # ---- head ----
        zs = work.tile([BSH, HID], F32R)
        zT = work.tile([128, 256], F32R)
        with tc.tile_pool(name="psum_z", bufs=1, space="PSUM") as pz_pool, \
             tc.tile_pool(name="psum_t2", bufs=1, space="PSUM") as pt2_pool:
            pz = pz_pool.tile([BSH, HID], F32)
            pT2 = pt2_pool.tile([128, 256], F32R)
            for nb in range(2):
                bank = pz[:, nb * 512:(nb + 1) * 512]
                for kc in range(4):
                    nc.tensor.matmul(
                        bank, lhsT=_r(hT[:, kc * 32:(kc + 1) * 32]),
                        rhs=_r(fc1t[:, kc * HID + nb * 512:
                                    kc * HID + (nb + 1) * 512]),
                        start=(kc == 0), stop=False)
                nc.tensor.matmul(
                    bank, lhsT=_r(ones[:]),
                    rhs=_r(fc1b[:, nb * 512:(nb + 1) * 512]),
                    start=False, stop=True)
            nc.scalar.activation(zs[:], pz[:], AFT.Relu)
            for j in range(8):
                nc.tensor.transpose(
                    _r(pT2[:, j * 32:(j + 1) * 32]),
                    _r(zs[:, j * 128:(j + 1) * 128]), _r(ident[:]))
            for j in range(8):
                nc.vector.tensor_copy(zT[:, j * 32:(j + 1) * 32],
                                      pT2[:, j * 32:(j + 1) * 32])

        with tc.tile_pool(name="fcw", bufs=3) as fcw_pool, \
             tc.tile_pool(name="fbw", bufs=2) as fbw_pool, \
             tc.tile_pool(name="outw", bufs=2) as out_pool, \
             tc.tile_pool(name="psum_w", bufs=2, space="PSUM") as pw_pool:
            for w in range(N_WIN):
                w0 = w * NW
                pw = pw_pool.tile([BSH, 4 * 512], F32)
                fbt = fbw_pool.tile([1, NW], F32R)
                nc.sync.dma_start(fbt[:], fc2b_d[:, w0:w0 + NW])
                for kc in range(8):
                    wt2 = fcw_pool.tile([128, NW], F32R, tag="fcw")
                    nc.sync.dma_start(wt2[:], fc2t_d[kc, :, w0:w0 + NW])
                    for nb in range(4):
                        nc.tensor.matmul(
                            pw[:, nb * 512: nb * 512 + NBANK],
                            lhsT=_r(zT[:, kc * 32:(kc + 1) * 32]),
                            rhs=_r(wt2[:, nb * NBANK:(nb + 1) * NBANK]),
                            start=(kc == 0), stop=False,
                            skip_group_check=True)
                for nb in range(4):
                    nc.tensor.matmul(
                        pw[:, nb * 512: nb * 512 + NBANK],
                        lhsT=_r(ones[:]),
                        rhs=_r(fbt[:, nb * NBANK:(nb + 1) * NBANK]),
                        start=False, stop=True, skip_group_check=True)
                ot = out_pool.tile([BSH, NW], F32)
                for nb in range(4):
                    nc.scalar.activation(
                        ot[:, nb * NBANK:(nb + 1) * NBANK],
                        pw[:, nb * 512: nb * 512 + NBANK], AFT.Copy)
                nc.sync.dma_start(out_d[:, w0:w0 + NW], ot[:])

    return nc


def _prep_host(x, hidden, trg, w_ih, w_hh, b_ih, b_hh, fc1_w, fc1_b, fc2_w,
               fc2_b, n_steps=T_STEPS):
    """Host-side weight permutation + per-core input maps."""
    f32 = np.float32
    w_hh = np.asarray(w_hh, f32)
    w_ih = np.asarray(w_ih, f32).reshape(-1)
    bias = (np.asarray(b_ih, f32) + np.asarray(b_hh, f32)).reshape(-1)
    # column permutation: new col b*512 + x*128 + j  <-  W row block_x*512 + b*128 + j
    # x in [i, f, o, g] -> torch row-blocks [0, 1, 3, 2]
    blk = np.array([0, 1, 3, 2])
    cols = (blk[:, None, None] * 512
            + np.arange(4)[None, :, None] * 128
            + np.arange(128)[None, None, :])          # [x, b, j]
    perm = np.transpose(cols, (1, 0, 2)).reshape(-1)  # b-major: [b, x, j]
    wt = np.ascontiguousarray(
        w_hh.T[:, perm].reshape(4, 128, G))           # [kc, p, col]
    wib = np.stack([w_ih[perm], bias[perm]])          # [2, G]

    fc1t = np.ascontiguousarray(np.asarray(fc1_w, f32).T.reshape(4, 128, HID))
    fc1bv = np.asarray(fc1_b, f32).reshape(1, HID)
    fc2t = np.ascontiguousarray(np.asarray(fc2_w, f32).T.reshape(8, 128, V))
    fc2bv = np.asarray(fc2_b, f32).reshape(1, V)

    trg_f = np.asarray(trg)[:, :n_steps].astype(f32)  # [B, n_steps]
    in_maps = []
    for c in range(N_CORES):
        sh = trg_f[c * BSH:(c + 1) * BSH]             # [BSH, n_steps]
        trga = np.empty((max(n_steps, 1), 2, BSH), f32)
        trga[:, 0, :] = sh.T
        trga[:, 1, :] = 1.0
        in_maps.append({
            "wt": wt, "wib": wib, "trga": trga,
            "fc1t": fc1t, "fc1b": fc1bv,
            "fc2t": fc2t, "fc2b": fc2bv,
            "ident": np.eye(32, dtype=f32),
            "onesr": np.ones((1, BSH), f32),
            "zi": np.zeros((128, 128), f32),
        })
    return in_maps


_CACHE = {}


def _get_program(n_steps=T_STEPS):
    if n_steps not in _CACHE:
        _CACHE[n_steps] = _build_program(n_steps)
    return _CACHE[n_steps]


def kernel(**inputs):
    nc = _get_program(T_STEPS)
    in_maps = _prep_host(**{k: inputs[k] for k in (
        "x", "hidden", "trg", "w_ih", "w_hh", "b_ih", "b_hh",
        "fc1_w", "fc1_b", "fc2_w", "fc2_b")}, n_steps=T_STEPS)
    res = run_bass_kernel_spmd(nc, in_maps, core_ids=list(range(N_CORES)))
    out = np.concatenate([res.results[c]["out"] for c in range(N_CORES)], axis=0)
    return out.astype(np.float32)


# revision 20
# speedup vs baseline: 17.1018x; 17.1018x over previous
"""Trainium2 Bass kernel for nn_Decoder (LSTM decoder + fc1/relu/fc2 head).

Strategy (8 NeuronCores, data-parallel over batch, 32 rows/core):
  - The 511-step LSTM recurrence runs fully in TRANSPOSED space: the
    state hT/cT live as [128 h-dims, 32 batch] column groups, the gate
    matmuls put the gate dimension on PSUM partitions (lhsT = static
    w_hh blocks, rhs = hT state slices), so no per-step transposes are
    ever needed. x*w_ih + bias enter via a K=2 matmul whose lhsT is a
    static [2,128] block ([w_ih; bias]) and rhs is a precomputed
    [trg_t; ones] column pair. The g-gate rows are pre-scaled by 2 on
    the host so ONE sigmoid activation evaluates all four gates
    (tanh(x) = 2*sigmoid(2x) - 1).
  - Matmuls use float32r (full-rate fp32 path on the PE).
  - Head: fc1+relu computed transposed the same way -> zT, then fc2
    streams the 131MB fc2_w.T from HBM in [128, 2000] tiles.
  - Host side: shard batch, pre-permute weights, concat core outputs.

Runtime note: on this runtime, cross-engine dependency hops and
ACT-engine ops are orders of magnitude more expensive than the cost
model predicts, so the design minimizes instruction count on ACT/DVE
and semaphore hops per step rather than PE streaming cycles.
"""

import sys

sys.path.insert(0, "/opt/trn_rl_repo")

import ml_dtypes
import numpy as np
from contextlib import ExitStack

import concourse.bass as bass
import concourse.mybir as mybir
import concourse.tile as tile
from concourse.bass_utils import run_bass_kernel_spmd

F32 = mybir.dt.float32
F32R = mybir.dt.float32r
BF16 = mybir.dt.bfloat16
AFT = mybir.ActivationFunctionType
ALU = mybir.AluOpType

N_CORES = 8
B = 256
BSH = B // N_CORES  # 32 batch rows per core
H = 512
G = 4 * H  # 2048
HID = 1024
V = 32000
T_STEPS = 511  # LSTM consumes trg[:, 0:511]

NW = 2000   # fc2 vocab window
NBANK = 500  # fc2 bank width (4 banks per window, 512-aligned in psum)
N_WIN = V // NW  # 16

_MAX_WAITS = 1


def _split_multi_waits(nc):
    """This walrus accepts at most one sync-wait per TPB instruction.
    Move extra waits onto same-engine nops placed directly before the
    instruction (engines execute their stream in order)."""
    ctr = 0
    for fn in nc.m.functions:
        for bb in fn.blocks:
            insts = list(bb.instructions)
            out = []
            changed = False
            for inst in insts:
                si = inst.sync_info
                if si is not None and si.on_wait and len(si.on_wait) > _MAX_WAITS:
                    waits = list(si.on_wait)
                    for w in waits[:-_MAX_WAITS]:
                        ctr += 1
                        nop = mybir.InstNoOp(
                            name=f"swsplit-{ctr}",
                            engine=inst.engine,
                            bass_nofuse=True,
                            sync_info=mybir.SyncInfo(on_wait=[w], on_update=[]),
                        )
                        nc.register_instruction(nop, overwrite=True)
                        out.append(nop)
                    si.on_wait = waits[-_MAX_WAITS:]
                    changed = True
                out.append(inst)
            if changed:
                bb.instructions = out


class _SplitDrainTileContext(tile.TileContext):
    def schedule_and_allocate(self):
        ret = super().schedule_and_allocate()
        _split_multi_waits(self.nc)
        return ret


def _build_program(n_steps=T_STEPS):
    nc = bass.Bass("TRN2", target_bir_lowering=False, debug=False, num_devices=1)
    ns = max(n_steps, 1)

    # DRAM inputs (f32r so DMA is a valid f32r producer for the PE)
    # wt2[p, (m*4+k)*128 + j] = w_hh_scaled[mrow(m,j), k*128+p]  (lhsT blocks)
    wt2_d = nc.dram_tensor("wt2", [128, 64 * 128], F32R, kind="ExternalInput").ap()
    wib2_d = nc.dram_tensor("wib2", [2, 16 * 128], F32R, kind="ExternalInput").ap()
    trga_d = nc.dram_tensor("trga", [2, ns * BSH], F32R, kind="ExternalInput").ap()
    fc1t2_d = nc.dram_tensor("fc1t2", [128, 32 * 128], F32R,
                             kind="ExternalInput").ap()
    fc1b2_d = nc.dram_tensor("fc1b2", [1, HID], F32R, kind="ExternalInput").ap()
    fc2t_d = nc.dram_tensor("fc2t", [8, 128, V], BF16, kind="ExternalInput").ap()
    fc2b_d = nc.dram_tensor("fc2b", [1, V], BF16, kind="ExternalInput").ap()
    onesb_d = nc.dram_tensor("onesb", [1, BSH], BF16, kind="ExternalInput").ap()
    ones_d = nc.dram_tensor("onesr", [1, BSH], F32R, kind="ExternalInput").ap()
    zi_d = nc.dram_tensor("zi", [128, 128], F32R, kind="ExternalInput").ap()
    out_d = nc.dram_tensor("out", [BSH, V], F32, kind="ExternalOutput").ap()

    with _SplitDrainTileContext(nc) as tc, ExitStack() as ctx:
        const = ctx.enter_context(tc.tile_pool(name="const", bufs=1))
        state = ctx.enter_context(tc.tile_pool(name="state", bufs=1))
        work = ctx.enter_context(tc.tile_pool(name="work", bufs=1))

        wt2 = const.tile([128, 64 * 128], F32R)
        nc.sync.dma_start(wt2[:], wt2_d[:])
        wib2 = const.tile([2, 16 * 128], F32R)
        nc.sync.dma_start(wib2[:], wib2_d[:])
        trgaux = const.tile([2, ns * BSH], F32R)
        nc.sync.dma_start(trgaux[:], trga_d[:])
        fc1t2 = const.tile([128, 32 * 128], F32R)
        nc.sync.dma_start(fc1t2[:], fc1t2_d[:])
        fc1b2 = const.tile([1, HID], F32R)
        nc.sync.dma_start(fc1b2[:], fc1b2_d[:])
        ones = const.tile([1, BSH], F32R)
        nc.sync.dma_start(ones[:], ones_d[:])
        ones_bf = const.tile([1, BSH], BF16)
        nc.sync.dma_start(ones_bf[:], onesb_d[:])

        # state, transposed space: col group b = h-chunk b ([128] x [32])
        cT = state.tile([128, 128], F32)
        hT = state.tile([128, 128], F32R)
        nc.vector.memset(cT[:], 0.0)
        nc.sync.dma_start(hT[:], zi_d[:])

        acts = work.tile([128, 512], F32R)  # sigmoid(gatesT): [i|f|o|sg] x4
        tg = work.tile([128, 128], F32)
        t1 = work.tile([128, 128], F32)
        sc = work.tile([128, 128], F32)
        tc_ = work.tile([128, 128], F32)

        aux0 = work.tile([2, BSH], F32R)
        aux1 = work.tile([2, BSH], F32R)

        def emit_step(xsl, pgT):
            for m in range(16):
                outm = pgT[:, m * 32:(m + 1) * 32]
                for k in range(4):
                    nc.tensor.matmul(
                        outm,
                        lhsT=wt2[:, (m * 4 + k) * 128:(m * 4 + k + 1) * 128],
                        rhs=hT[:, k * 32:(k + 1) * 32],
                        start=(k == 0), stop=False)
                nc.tensor.matmul(
                    outm, lhsT=wib2[:, m * 128:(m + 1) * 128],
                    rhs=xsl, start=False, stop=True)

            # one sigmoid for all gates (g pre-scaled by 2 on host)
            nc.scalar.activation(acts[:], pgT[:], AFT.Sigmoid)

            # T-space algebra ([128,128] each; chunk b at cols b*32)
            nc.vector.tensor_scalar(tg[:], acts[:, 384:512], 2.0, 1.0,
                                    ALU.mult, ALU.subtract)
            nc.vector.tensor_mul(t1[:], acts[:, 0:128], tg[:])
            nc.vector.tensor_mul(cT[:], cT[:], acts[:, 128:256])
            nc.vector.tensor_add(cT[:], cT[:], t1[:])
            nc.scalar.activation(sc[:], cT[:], AFT.Sigmoid, scale=2.0)
            nc.vector.tensor_scalar(tc_[:], sc[:], 2.0, 1.0,
                                    ALU.mult, ALU.subtract)
            nc.vector.tensor_mul(hT[:], acts[:, 256:384], tc_[:])

        with tc.tile_pool(name="psum_g", bufs=1, space="PSUM") as pg_pool:
            pgT0 = pg_pool.tile([128, 512], F32, tag="pg0", name="pgT0")
            pgT1 = pg_pool.tile([128, 512], F32, tag="pg1", name="pgT1")
            # t = 0 prologue (static), then 255 x 2-step hardware loop
            emit_step(trgaux[:, 0:BSH], pgT0)
            if n_steps > 1:
                assert n_steps == 511
                with tc.For_i(1, n_steps, 2) as tv:
                    off = tv * BSH
                    nc.vector.tensor_copy(aux0[:], trgaux[:, bass.ds(off, BSH)])
                    emit_step(aux0[:], pgT1)
                    nc.vector.tensor_copy(aux1[:],
                                          trgaux[:, bass.ds(off + BSH, BSH)])
                    emit_step(aux1[:], pgT0)

        # ---- head: fc1 transposed (zT directly), then fc2 ----
        zT = work.tile([128, 256], BF16)
        with tc.tile_pool(name="psum_z", bufs=1, space="PSUM") as pz_pool:
            pzT = pz_pool.tile([128, 256], F32)  # 8 m-chunks x 32
            for m in range(8):
                outm = pzT[:, m * 32:(m + 1) * 32]
                for k in range(4):
                    nc.tensor.matmul(
                        outm,
                        lhsT=fc1t2[:, (m * 4 + k) * 128:(m * 4 + k + 1) * 128],
                        rhs=hT[:, k * 32:(k + 1) * 32],
                        start=(k == 0), stop=False)
                nc.tensor.matmul(
                    outm, lhsT=fc1b2[:, m * 128:(m + 1) * 128],
                    rhs=ones[:], start=False, stop=True)
            nc.scalar.activation(zT[:], pzT[:], AFT.Relu)

        with tc.tile_pool(name="fcw", bufs=3) as fcw_pool, \
             tc.tile_pool(name="fbw", bufs=2) as fbw_pool, \
             tc.tile_pool(name="outw", bufs=2) as out_pool, \
             tc.tile_pool(name="psum_w", bufs=2, space="PSUM") as pw_pool:
            for w in range(N_WIN):
                w0 = w * NW
                pw = pw_pool.tile([BSH, 4 * 512], F32)
                fbt = fbw_pool.tile([1, NW], BF16)
                nc.sync.dma_start(fbt[:], fc2b_d[:, w0:w0 + NW])
                for kc in range(8):
                    wt_f = fcw_pool.tile([128, NW], BF16, tag="fcw")
                    nc.sync.dma_start(wt_f[:], fc2t_d[kc, :, w0:w0 + NW])
                    for nb in range(4):
                        nc.tensor.matmul(
                            pw[:, nb * 512: nb * 512 + NBANK],
                            lhsT=zT[:, kc * 32:(kc + 1) * 32],
                            rhs=wt_f[:, nb * NBANK:(nb + 1) * NBANK],
                            start=(kc == 0), stop=False,
                            skip_group_check=True)
                for nb in range(4):
                    nc.tensor.matmul(
                        pw[:, nb * 512: nb * 512 + NBANK],
                        lhsT=ones_bf[:],
                        rhs=fbt[:, nb * NBANK:(nb + 1) * NBANK],
                        start=False, stop=True, skip_group_check=True)
                ot = out_pool.tile([BSH, NW], F32)
                for nb in range(4):
                    nc.scalar.activation(
                        ot[:, nb * NBANK:(nb + 1) * NBANK],
                        pw[:, nb * 512: nb * 512 + NBANK], AFT.Copy)
                nc.sync.dma_start(out_d[:, w0:w0 + NW], ot[:])

    return nc


def _prep_host(x, hidden, trg, w_ih, w_hh, b_ih, b_hh, fc1_w, fc1_b, fc2_w,
               fc2_b, n_steps=T_STEPS):
    """Host-side weight permutation + per-core input maps."""
    f32 = np.float32
    ns = max(n_steps, 1)
    w_hh = np.asarray(w_hh, f32)
    w_ih = np.asarray(w_ih, f32).reshape(-1)
    bias = (np.asarray(b_ih, f32) + np.asarray(b_hh, f32)).reshape(-1)

    # m-chunk order: [i0..3, f0..3, o0..3, g0..3]; torch row blocks i,f,g,o
    blkmap = np.array([0, 1, 3, 2])  # i,f,o,g -> torch block index
    mrows = np.concatenate([
        blkmap[gt] * 512 + hc * 128 + np.arange(128)
        for gt in range(4) for hc in range(4)])          # [2048] W row ids
    scale = np.where(np.arange(16 * 128) >= 12 * 128, 2.0, 1.0).astype(f32)

    wsc = w_hh[mrows] * scale[:, None]                   # [2048, 512]
    w4 = wsc.reshape(16, 128, 4, 128)                    # [m, j, k, p]
    wt2 = np.ascontiguousarray(
        np.transpose(w4, (3, 0, 2, 1)).reshape(128, 64 * 128))

    wib2 = np.stack([w_ih[mrows] * scale, bias[mrows] * scale])  # [2, 2048]

    fc1_w = np.asarray(fc1_w, f32)
    f4 = fc1_w.reshape(8, 128, 4, 128)                   # [m, j, k, p]
    fc1t2 = np.ascontiguousarray(
        np.transpose(f4, (3, 0, 2, 1)).reshape(128, 32 * 128))
    fc1b2 = np.asarray(fc1_b, f32).reshape(1, HID)

    bf16 = ml_dtypes.bfloat16
    fc2t = np.ascontiguousarray(
        np.asarray(fc2_w, f32).T.reshape(8, 128, V).astype(bf16))
    fc2bv = np.asarray(fc2_b, f32).reshape(1, V).astype(bf16)

    trg_f = np.asarray(trg)[:, :n_steps].astype(f32)     # [B, n_steps]
    in_maps = []
    for c in range(N_CORES):
        sh = trg_f[c * BSH:(c + 1) * BSH]                # [BSH, n_steps]
        trga = np.ones((2, ns * BSH), f32)
        trga[0, :n_steps * BSH] = sh.T.reshape(-1)
        in_maps.append({
            "wt2": wt2, "wib2": wib2, "trga": trga,
            "fc1t2": fc1t2, "fc1b2": fc1b2,
            "fc2t": fc2t, "fc2b": fc2bv,
            "onesr": np.ones((1, BSH), f32),
            "onesb": np.ones((1, BSH), bf16),
            "zi": np.zeros((128, 128), f32),
        })
    return in_maps


_CACHE = {}


def _get_program(n_steps=T_STEPS):
    if n_steps not in _CACHE:
        _CACHE[n_steps] = _build_program(n_steps)
    return _CACHE[n_steps]


def kernel(**inputs):
    nc = _get_program(T_STEPS)
    in_maps = _prep_host(**{k: inputs[k] for k in (
        "x", "hidden", "trg", "w_ih", "w_hh", "b_ih", "b_hh",
        "fc1_w", "fc1_b", "fc2_w", "fc2_b")}, n_steps=T_STEPS)
    res = run_bass_kernel_spmd(nc, in_maps, core_ids=list(range(N_CORES)))
    out = np.concatenate([res.results[c]["out"] for c in range(N_CORES)], axis=0)
    return out.astype(np.float32)
